# revision 1
# baseline (speedup 1.0000x reference)
"""LocationSensitiveSoftAttention on 8 Trainium2 NeuronCores (Bass/Tile).

Contract: kernel(**inputs) takes the FULL unsharded inputs (numpy arrays, keys
as in setup_inputs()) and returns the FULL output [64, 1, 256] fp32.

Strategy: data-parallel over batch B=64 -> 8 batches per core; weights
replicated. Math restructure (exact up to fp rounding):
  pre[b,t,:] = memory[b,t,:] @ (Wm@We) + f[b,:,t] @ (Wl@We)
               + (query[b,1]@Wq + bq + bm + bl) @ We + be + conv_b @ (Wl@We)
  h = tanh(pre); energy = h @ v_a; s = sigmoid(energy)
  w = state + s/sum(s)    (cumulate)
  context = (w @ memory) @ Wm + (sum(state) + 1) * bm
The last line pulls the memory projection out of the weighted time-sum, so
mem_proj [B,T,U] is never materialized.
"""

import sys

for _p in ("/root/.axon_site", "/root/.axon_site/_ro/trn_rl_repo",
           "/root/.axon_site/_ro/pypackages", "/opt/trn_rl_repo"):
    if _p not in sys.path:
        sys.path.append(_p)

import numpy as np
import ml_dtypes

B, TQ, T = 64, 2, 2048
HID, ENC, U, FILT, K = 1024, 512, 256, 32, 31
N_CORES = 8
PB = B // N_CORES  # batches per core
PAD = K // 2  # 15
NT = T // 128  # 16 t-tiles
NBLK = T // 512  # 4 t-blocks

BF16 = ml_dtypes.bfloat16

_BUILT = {}
TRACE = False
LAST_RESULTS = None


def _build_nc(repeat=1):
    import concourse.bacc as bacc
    import concourse.mybir as mybir
    import concourse.tile as tile
    import concourse.bass as bass

    f32 = mybir.dt.float32
    bf16 = mybir.dt.bfloat16
    AF = mybir.ActivationFunctionType
    ALU = mybir.AluOpType
    AX = mybir.AxisListType

    nc = bacc.Bacc("TRN2", target_bir_lowering=False, debug=False,
                   num_devices=N_CORES)

    # ---- DRAM I/O ----
    mem_d = nc.dram_tensor("mem", [PB, T, ENC], bf16, kind="ExternalInput")
    memt_d = nc.dram_tensor("memt", [PB, 128, NT * 4, 128], bf16, kind="ExternalInput")
    spad_d = nc.dram_tensor("spad", [PB, T + 2 * PAD], f32, kind="ExternalInput")
    spadb_d = nc.dram_tensor("spadb", [PB, T + 2 * PAD], bf16, kind="ExternalInput")
    q1_d = nc.dram_tensor("q1", [PB, HID], f32, kind="ExternalInput")
    wmwe_d = nc.dram_tensor("wmwe", [ENC, U], bf16, kind="ExternalInput")
    wlwe_d = nc.dram_tensor("wlwe", [FILT, U], bf16, kind="ExternalInput")
    c0_d = nc.dram_tensor("c0", [1, U], bf16, kind="ExternalInput")
    convwT_d = nc.dram_tensor("convwT", [K, FILT], bf16, kind="ExternalInput")
    wq_d = nc.dram_tensor("wq", [HID, U], bf16, kind="ExternalInput")
    we_d = nc.dram_tensor("we", [U, U], bf16, kind="ExternalInput")
    wm_d = nc.dram_tensor("wm", [ENC, U], f32, kind="ExternalInput")
    bm_d = nc.dram_tensor("bm", [1, U], f32, kind="ExternalInput")
    vaT_d = nc.dram_tensor("vaT", [128, 2], bf16, kind="ExternalInput")
    idf_d = nc.dram_tensor("idf", [128, 128], f32, kind="ExternalInput")
    idb_d = nc.dram_tensor("idb", [128, 128], bf16, kind="ExternalInput")
    out_d = nc.dram_tensor("out", [PB, U], f32, kind="ExternalOutput")

    with tile.TileContext(nc) as tc:
        with (
            tc.tile_pool(name="consts", bufs=1) as consts,
            tc.tile_pool(name="nat", bufs=3) as natp,
            tc.tile_pool(name="memT", bufs=3) as mtp,
            tc.tile_pool(name="hb", bufs=4) as hbp,
            tc.tile_pool(name="rows", bufs=2) as rowp,
            tc.tile_pool(name="convp", bufs=2) as convp,
            tc.tile_pool(name="psA", bufs=4, space="PSUM") as psA,
            tc.tile_pool(name="psB", bufs=1, space="PSUM") as psB,
            tc.tile_pool(name="psC", bufs=2, space="PSUM") as psC,
        ):
          def _body():
              # ---- load constants ----
              wmwe_sb = []
              for ec in range(4):
                  t_ = consts.tile([128, U], bf16, tag=f"wmwe{ec}")
                  nc.scalar.dma_start(out=t_[:], in_=wmwe_d.ap()[ec * 128:(ec + 1) * 128, :])
                  wmwe_sb.append(t_)
              wq_sb = []
              for j in range(8):
                  t_ = consts.tile([128, U], bf16, tag=f"wq{j}")
                  nc.scalar.dma_start(out=t_[:], in_=wq_d.ap()[j * 128:(j + 1) * 128, :])
                  wq_sb.append(t_)
              we_sb = []
              for j in range(2):
                  t_ = consts.tile([128, U], bf16, tag=f"we{j}")
                  nc.scalar.dma_start(out=t_[:], in_=we_d.ap()[j * 128:(j + 1) * 128, :])
                  we_sb.append(t_)
              wm_sb = []
              for ec in range(4):
                  t_ = consts.tile([128, U], f32, tag=f"wm{ec}")
                  nc.scalar.dma_start(out=t_[:], in_=wm_d.ap()[ec * 128:(ec + 1) * 128, :])
                  wm_sb.append(t_)
              wlwe_sb = consts.tile([FILT, U], bf16, tag="wlwe")
              nc.scalar.dma_start(out=wlwe_sb[:], in_=wlwe_d.ap())
              c0_sb = consts.tile([1, U], bf16, tag="c0")
              nc.scalar.dma_start(out=c0_sb[:], in_=c0_d.ap())
              convwT_sb = consts.tile([K, FILT], bf16, tag="convwT")
              nc.scalar.dma_start(out=convwT_sb[:], in_=convwT_d.ap())
              bm_sb = consts.tile([1, U], f32, tag="bm")
              nc.scalar.dma_start(out=bm_sb[:], in_=bm_d.ap())
              vaT_sb = consts.tile([128, 2], bf16, tag="vaT")
              nc.scalar.dma_start(out=vaT_sb[:], in_=vaT_d.ap())
              idf_sb = consts.tile([128, 128], f32, tag="idf")
              nc.scalar.dma_start(out=idf_sb[:], in_=idf_d.ap())
              idb_sb = consts.tile([128, 128], bf16, tag="idb")
              nc.scalar.dma_start(out=idb_sb[:], in_=idb_d.ap())
              state_sb = consts.tile([PB, T + 2 * PAD], f32, tag="state")
              nc.scalar.dma_start(out=state_sb[:], in_=spad_d.ap())
              ones8 = consts.tile([1, 8], bf16, tag="ones8")
              nc.vector.memset(ones8[:], 1.0)
              call_sb = consts.tile([PB, ENC], f32, tag="call")

              # ---- sum(state) + 1 row [1, 8] ----
              stsum = consts.tile([PB, 1], f32, tag="stsum")
              nc.vector.tensor_reduce(stsum[:], state_sb[:, PAD:PAD + T],
                                      axis=AX.X, op=ALU.add)
              ps_sig = psC.tile([1, 8], f32, tag="misc")
              nc.tensor.matmul(ps_sig[:], stsum[:], idf_sb[0:PB, 0:PB],
                               is_transpose=True)
              sig_row = consts.tile([1, 8], f32, tag="sigrow")
              nc.vector.tensor_scalar_add(sig_row[:], ps_sig[:], 1.0)

              # ---- pq -> r rows [8, 256] bf16 ----
              q1_sb = consts.tile([PB, HID], f32, tag="q1")
              nc.scalar.dma_start(out=q1_sb[:], in_=q1_d.ap())
              q1_bf = consts.tile([16, HID], bf16, tag="q1bf")
              nc.vector.memset(q1_bf[:], 0.0)
              nc.vector.tensor_copy(q1_bf[0:PB, :], q1_sb[:])
              q1T2 = consts.tile([128, 8, 16], bf16, tag="q1T2")
              nc.sync.dma_start(out=q1T2[:], in_=q1_bf[:], transpose=True)
              pq_ps = psC.tile([PB, U], f32, tag="misc")
              for j in range(8):
                  nc.tensor.matmul(pq_ps[:], q1T2[:, j, 0:PB], wq_sb[j][:],
                                   start=(j == 0), stop=(j == 7))
              pq_bf = consts.tile([16, U], bf16, tag="pqbf")
              nc.vector.memset(pq_bf[:], 0.0)
              nc.scalar.activation(pq_bf[0:PB, :], pq_ps[:], AF.Copy)
              pqT2 = consts.tile([128, 2, 16], bf16, tag="pqT2")
              nc.sync.dma_start(out=pqT2[:], in_=pq_bf[:], transpose=True)
              r_ps = psC.tile([PB, U], f32, tag="misc")
              nc.tensor.matmul(r_ps[:], pqT2[:, 0, 0:PB], we_sb[0][:],
                               start=True, stop=False)
              nc.tensor.matmul(r_ps[:], pqT2[:, 1, 0:PB], we_sb[1][:],
                               start=False, stop=False)
              nc.tensor.matmul(r_ps[:], ones8[:], c0_sb[:], start=False, stop=True)
              r_bf = consts.tile([PB, U], bf16, tag="rbf")
              nc.scalar.activation(r_bf[:], r_ps[:], AF.Copy)

              # ---- per-batch main loop (software-pipelined loads) ----
              def load_b(b):
                  st = {}
                  # memory tiles (cast to bf16 during SWDGE DMA):
                  # nat[p, ti, e] = mem[b, ti*128+p, e]
                  nat = natp.tile([128, NT, ENC], bf16, tag="nat", name=f"nat{b}")
                  memT2 = mtp.tile([128, NT * 4, 128], bf16, tag="memT2",
                                   name=f"memT2_{b}")
                  mem_b = bass.AP(
                      tensor=mem_d, offset=b * T * ENC,
                      ap=[[ENC, 128], [128 * ENC, NT], [1, ENC]])
                  nc.sync.dma_start(out=nat[:], in_=mem_b)
                  # pre-transposed on host:
                  # memT2[ew, ti*4+ec, tw] = mem[b, ti*128+tw, ec*128+ew]
                  nc.sync.dma_start(out=memT2[:], in_=memt_d.ap()[b])
                  # conv inputs: 31 overlapping shifted copies of padded state
                  shifted = convp.tile([K, T], bf16, tag="shifted",
                                       name=f"shifted{b}", bufs=3)
                  src = bass.AP(tensor=spadb_d, offset=b * (T + 2 * PAD),
                                ap=[[1, K], [1, T]])
                  nc.sync.dma_start(out=shifted[:], in_=src)
                  faug = convp.tile([FILT + 1, T], bf16, tag="faug",
                                    name=f"faug{b}", bufs=3)
                  nc.vector.memset(faug[FILT:FILT + 1, :], 1.0)
                  waug = convp.tile([FILT + 1, U], bf16, tag="waug",
                                    name=f"waug{b}", bufs=3)
                  nc.vector.tensor_copy(waug[0:FILT, :], wlwe_sb[:])
                  nc.sync.dma_start(out=waug[FILT:FILT + 1, :],
                                    in_=r_bf[b:b + 1, :])
                  srow_b = rowp.tile([1, T], f32, tag="srowb", name=f"srowb{b}", bufs=3)
                  nc.sync.dma_start(out=srow_b[:],
                                    in_=spad_d.ap()[b, PAD:PAD + T])
                  st.update(nat=nat, memT2=memT2, shifted=shifted, faug=faug,
                            waug=waug, srow_b=srow_b)
                  return st

              def compute_b(b, st):
                  nat, memT2 = st["nat"], st["memT2"]
                  shifted, faug, waug = st["shifted"], st["faug"], st["waug"]
                  for tb in range(NBLK):
                      f_ps = psB.tile([FILT, 512], f32, tag="fps")
                      nc.tensor.matmul(f_ps[:], convwT_sb[:],
                                       shifted[:, tb * 512:(tb + 1) * 512],
                                       start=True, stop=True)
                      nc.vector.tensor_copy(faug[0:FILT, tb * 512:(tb + 1) * 512],
                                            f_ps[:])
                  s_row = rowp.tile([1, T], bf16, tag="srow")
                  ssum4 = rowp.tile([1, NBLK], f32, tag="ssum4")
                  for tb in range(NBLK):
                      h_bf = []
                      for vch in range(2):
                          pre_ps = psA.tile([128, 512], f32, tag="pre")
                          for ec in range(4):
                              # moving = [128e, 4 t-tiles x 128] for (tb, ec)
                              mov = memT2[:, bass.ds(tb * 16 + ec, 1), :]
                              mov = bass.AP(
                                  tensor=mov.tensor, offset=mov.offset,
                                  ap=[mov.ap[0], [4 * 128, 4], [1, 128]])
                              nc.tensor.matmul(
                                  pre_ps[:],
                                  wmwe_sb[ec][:, vch * 128:(vch + 1) * 128],
                                  mov,
                                  start=(ec == 0), stop=False)
                          nc.tensor.matmul(
                              pre_ps[:],
                              waug[:, vch * 128:(vch + 1) * 128],
                              faug[:, tb * 512:(tb + 1) * 512],
                              start=False, stop=True)
                          hb = hbp.tile([128, 512], bf16, tag=f"h{vch}")
                          nc.scalar.activation(hb[:], pre_ps[:], AF.Tanh)
                          h_bf.append(hb)
                      en_ps = psB.tile([1, 512], f32, tag="enps", bufs=1)
                      nc.tensor.matmul(en_ps[:], vaT_sb[:, 0:1], h_bf[0][:],
                                       start=True, stop=False)
                      nc.tensor.matmul(en_ps[:], vaT_sb[:, 1:2], h_bf[1][:],
                                       start=False, stop=True)
                      # s = sigmoid(energy) with per-block partial sums
                      nc.scalar.activation(s_row[:, tb * 512:(tb + 1) * 512],
                                           en_ps[:], AF.Sigmoid,
                                           accum_out=ssum4[:, tb:tb + 1])

                  ssum = rowp.tile([1, 1], f32, tag="ssum")
                  nc.vector.tensor_reduce(ssum[:], ssum4[:], axis=AX.X, op=ALU.add)
                  rec = rowp.tile([1, 1], f32, tag="rec")
                  nc.vector.reciprocal(rec[:], ssum[:])
                  wrow = rowp.tile([1, T], bf16, tag="wrow")
                  nc.vector.scalar_tensor_tensor(
                      wrow[:], in0=s_row[:], scalar=rec[:],
                      in1=st["srow_b"][:],
                      op0=ALU.mult, op1=ALU.add)
                  w2d = rowp.tile([NT, 128], bf16, tag="w2d")
                  nc.sync.dma_start(out=w2d[:], in_=wrow[:])
                  wT_ps = psC.tile([128, NT], bf16, tag="misc")
                  nc.tensor.matmul(wT_ps[:], w2d[:], idb_sb[0:NT, 0:NT],
                                   is_transpose=True)
                  wT_sb = rowp.tile([128, NT], bf16, tag="wT")
                  nc.vector.tensor_copy(wT_sb[:], wT_ps[:])

                  # cvec = w @ memory  [1, 512]
                  cv_ps = psC.tile([1, ENC], f32, tag="misc")
                  for ch in range(NT):
                      nc.tensor.matmul(cv_ps[:], wT_sb[:, ch:ch + 1],
                                       nat[:, ch, :],
                                       start=(ch == 0), stop=(ch == NT - 1))
                  cv_sb = rowp.tile([1, ENC], f32, tag="cvsb")
                  nc.vector.tensor_copy(cv_sb[:], cv_ps[:])
                  nc.sync.dma_start(out=call_sb[b:b + 1, :], in_=cv_sb[:])

              st = load_b(0)
              for b in range(PB):
                  st_next = load_b(b + 1) if b + 1 < PB else None
                  compute_b(b, st)
                  st = st_next

              # ---- final: context = Call @ Wm + sig_row^T * bm ----
              callT = []
              for ch in range(4):
                  pst = psC.tile([128, PB], f32, tag="misc")
                  nc.tensor.matmul(pst[:], call_sb[:, ch * 128:(ch + 1) * 128],
                                   idf_sb[0:PB, 0:PB], is_transpose=True)
                  t_ = consts.tile([128, PB], f32, tag=f"callT{ch}")
                  nc.vector.tensor_copy(t_[:], pst[:])
                  callT.append(t_)
              ctx_ps = psC.tile([PB, U], f32, tag="misc")
              for ch in range(4):
                  nc.tensor.matmul(ctx_ps[:], callT[ch][:], wm_sb[ch][:],
                                   start=(ch == 0), stop=False)
              sig_row_f = sig_row
              nc.tensor.matmul(ctx_ps[:], sig_row_f[:], bm_sb[:],
                               start=False, stop=True)
              ctx_sb = consts.tile([PB, U], f32, tag="ctx")
              nc.vector.tensor_copy(ctx_sb[:], ctx_ps[:])
              nc.sync.dma_start(out=out_d.ap(), in_=ctx_sb[:])

          for _rep in range(repeat):
              _body()
    nc.compile()
    return nc


def _host_prep(inputs):
    """Fold weights on host (weight-only transforms) and shard per core."""
    f32 = np.float32
    Wq = np.asarray(inputs["Wq"], f32)
    bq = np.asarray(inputs["bq"], f32)
    Wm = np.asarray(inputs["Wm"], f32)
    bm = np.asarray(inputs["bm"], f32)
    Wl = np.asarray(inputs["Wl"], f32)
    bl = np.asarray(inputs["bl"], f32)
    conv_w = np.asarray(inputs["conv_w"], f32)
    conv_b = np.asarray(inputs["conv_b"], f32)
    We = np.asarray(inputs["We"], f32)
    be = np.asarray(inputs["be"], f32)
    v_a = np.asarray(inputs["v_a"], f32)

    WmWe = (Wm @ We).astype(f32)
    WlWe = (Wl @ We).astype(f32)
    c0 = ((bq + bm + bl) @ We + be + conv_b @ WlWe).astype(f32)

    query = np.asarray(inputs["query"], f32)
    state = np.asarray(inputs["state"], f32)
    memory = np.ascontiguousarray(np.asarray(inputs["memory"], f32))

    spad = np.zeros((B, T + 2 * PAD), f32)
    spad[:, PAD:PAD + T] = state
    q1 = np.ascontiguousarray(query[:, 1, :])

    ident = np.eye(128, dtype=f32)
    shared = {
        "wmwe": WmWe.astype(BF16),
        "wlwe": WlWe.astype(BF16),
        "c0": c0.reshape(1, U).astype(BF16),
        "convwT": np.ascontiguousarray(conv_w[:, 0, :].T).astype(BF16),
        "wq": Wq.astype(BF16),
        "we": We.astype(BF16),
        "wm": Wm.astype(f32),
        "bm": bm.reshape(1, U).astype(f32),
        "vaT": np.ascontiguousarray(v_a.reshape(2, 128).T).astype(BF16),
        "idf": ident,
        "idb": ident.astype(BF16),
    }
    in_maps = []
    for c in range(N_CORES):
        sl = slice(c * PB, (c + 1) * PB)
        m = dict(shared)
        mb = np.ascontiguousarray(memory[sl]).astype(BF16)
        m["mem"] = mb
        m["memt"] = np.ascontiguousarray(
            mb.reshape(PB, NT, 128, 4, 128).transpose(0, 4, 1, 3, 2)
            .reshape(PB, 128, NT * 4, 128))
        m["spad"] = np.ascontiguousarray(spad[sl])
        m["spadb"] = np.ascontiguousarray(spad[sl]).astype(BF16)
        m["q1"] = np.ascontiguousarray(q1[sl])
        in_maps.append(m)
    return in_maps


def kernel(**inputs) -> np.ndarray:
    global LAST_RESULTS
    from concourse import bass_utils

    if "nc" not in _BUILT:
        _BUILT["nc"] = _build_nc()
    nc = _BUILT["nc"]

    in_maps = _host_prep(inputs)
    res = bass_utils.run_bass_kernel_spmd(
        nc, in_maps, core_ids=list(range(N_CORES)), trace=TRACE)
    LAST_RESULTS = res
    out = np.concatenate([res.results[c]["out"] for c in range(N_CORES)], axis=0)
    return out.reshape(B, 1, U).astype(np.float32)

